# revision 12
# baseline (speedup 1.0000x reference)
"""AnomalyAttention Trainium2 kernel — 8 NeuronCores, data-parallel over batch.

Computes, for B=32, L=512, H=8, E=64 (shapes hardcoded):
    scores   = Q @ K^T (causal masked), series = softmax(scores/8)
    V_out    = series @ V
    prior    = 1/(sqrt(2pi) sig) * exp(-(i-j)^2 / (2 sig^2))
    sigma_out= broadcast(sig)  with sig = 3^(sigmoid(5*sigma)+1e-5) - 1

Each of the 8 cores handles 4 batches; all heads computed locally, no
collectives. The host pre-transposes Q/K to [E, L] layout and casts Q/K/V to
bf16 (TensorEngine compute dtype), and appends a ones-column per head to V so
the series@V matmul also produces the softmax row-sums. On-chip: QK^T and
series@V on the TensorEngine (fp32 accumulation), exp/prior on the
ScalarEngine, normalization/masking/sigma-broadcast on the VectorEngine.
Input slabs load through the gpsimd software-DGE queue so they never sit
behind the output stream on the two hardware DMA queues (SP and ACT).
sigma_out is written with 4 rows packed per SBUF partition, which makes the
DMA destination runs 8 KiB instead of 2 KiB. The causal structure skips all
fully-masked score blocks; their series output stays zero because the runner
donates pre-zeroed output buffers.
"""
import math
import sys
import types
from contextlib import ExitStack

sys.path.insert(0, "/opt/trn_rl_repo")

import numpy as np

# NTFF profile hook shim: the container's antenv package lacks axon_hooks, so
# register an equivalent module before concourse imports it (trace=True path).
if "antenv.axon_hooks" not in sys.modules:
    _hook_mod = types.ModuleType("antenv.axon_hooks")
    _hook_store = [None]
    _hook_mod.set_axon_ntff_profile_hook = lambda h: _hook_store.__setitem__(0, h)
    _hook_mod.get_axon_ntff_profile_hook = lambda: _hook_store[0]
    sys.modules["antenv.axon_hooks"] = _hook_mod
    try:
        import antenv

        antenv.axon_hooks = _hook_mod
        from trn_agent_boot.trn_boot import _ntff_profile_via_ctypes

        _hook = _ntff_profile_via_ctypes("/opt/axon/libaxon_pjrt.so")
        if _hook is not None:
            _hook_mod.set_axon_ntff_profile_hook(_hook)
    except Exception:
        pass

import ml_dtypes
import concourse.bass as bass
import concourse.tile as tile
from concourse import mybir
from concourse.bass_utils import run_bass_kernel_spmd
from concourse.masks import make_identity

F32 = mybir.dt.float32
BF16 = mybir.dt.bfloat16
ACT = mybir.ActivationFunctionType
MUL = mybir.AluOpType.mult

N_CORES = 8
B, L, H, E = 32, 512, 8, 64
BL = B // N_CORES  # batches per core
P = 128
NB = L // P  # 4 row blocks
RPP = L // P  # rows packed per partition for sigma_out (4)
HP = H // 2  # head pairs (two heads' E dims stacked on 128 partitions)
E1 = E + 1  # V columns per head incl. the ones column
LN3 = math.log(3.0)
NEG_HALF_LN_2PI = -0.5 * math.log(2.0 * math.pi)


def _split_excess_waits(nc):
    """This container's walrus accepts at most one sync-wait per instruction
    (two for EventSemaphore), but Tile attaches one wait per dependency.
    Hoist excess waits onto dedicated same-engine NOPs placed immediately
    before the instruction — equivalent for monotone (sem-ge) waits."""
    fixn = [0]
    for f in nc.m.functions:
        for bb in f.blocks:
            out = []
            changed = False
            for inst in bb.instructions:
                si = inst.sync_info
                n = len(si.on_wait) if si and si.on_wait else 0
                cap = 2 if isinstance(inst, mybir.InstEventSemaphore) else 1
                if n > cap:
                    waits = list(si.on_wait)
                    # keep non-monotone (eq) waits on the instruction itself
                    waits.sort(key=lambda w: "ge" in w.wait_mode)
                    keep, hoist = waits[:cap], waits[cap:]
                    for w in hoist:
                        assert "ge" in w.wait_mode, w
                        fixn[0] += 1
                        nop = mybir.InstNoOp(
                            name=f"Iwfix-{fixn[0]}",
                            engine=inst.engine,
                            ins=[],
                            outs=[],
                            bass_nofuse=True,
                        )
                        nop.sync_info = type(si)(on_wait=[w], on_update=[])
                        out.append(nop)
                    si.on_wait = keep
                    changed = True
                out.append(inst)
            if changed:
                bb.instructions = out
    return nc


def _bcast(ap, n):
    """Append a stride-0 dimension of size n to an AP (free-dim broadcast)."""
    return bass.AP(tensor=ap.tensor, offset=ap.offset, ap=[*ap.ap, [0, n]])


def _build():
    nc = bass.Bass("TRN2")
    qt = nc.declare_dram_parameter("qt", [BL, HP, P, L], BF16, isOutput=False)
    kt = nc.declare_dram_parameter("kt", [BL, HP, P, L], BF16, isOutput=False)
    v5 = nc.declare_dram_parameter("v5", [BL, L, H * E1], BF16, isOutput=False)
    sg = nc.declare_dram_parameter("sg", [BL, L, H], F32, isOutput=False)
    sgp = nc.declare_dram_parameter("sgp", [BL, P, RPP, H], F32, isOutput=False)
    d2 = nc.declare_dram_parameter("d2", [L, L], F32, isOutput=False)
    vo = nc.declare_dram_parameter("vo", [BL, L, H * E], F32, isOutput=True)
    so = nc.declare_dram_parameter("so", [BL, H, L, L], F32, isOutput=True)
    po = nc.declare_dram_parameter("po", [BL, H, L, L], F32, isOutput=True)
    go = nc.declare_dram_parameter("go", [BL, H, L, L], F32, isOutput=True)

    with ExitStack() as ctx:
        tc = ctx.enter_context(tile.TileContext(nc))
        consts = ctx.enter_context(tc.tile_pool(name="consts", bufs=1))
        sparams = ctx.enter_context(tc.tile_pool(name="sparams", bufs=1))
        slabs = ctx.enter_context(tc.tile_pool(name="slabs", bufs=2))
        work = ctx.enter_context(tc.tile_pool(name="work", bufs=5))
        gop = ctx.enter_context(tc.tile_pool(name="gop", bufs=3))
        eTp = ctx.enter_context(tc.tile_pool(name="eTp", bufs=10))
        small = ctx.enter_context(tc.tile_pool(name="small", bufs=12))
        ps_sc = ctx.enter_context(tc.tile_pool(name="ps_sc", bufs=3, space="PSUM"))
        ps_tr = ctx.enter_context(tc.tile_pool(name="ps_tr", bufs=3, space="PSUM"))
        ps_av = ctx.enter_context(tc.tile_pool(name="ps_av", bufs=2, space="PSUM"))

        ident = consts.tile([P, P], BF16)
        make_identity(nc, ident)
        # multiplicative causal masks: trimask[q,s]=1 iff s<=q (diag block of
        # the [q,s] layout); trimaskT[s,q]=1 iff s<=q (its transpose)
        trimask = consts.tile([P, P], BF16)
        nc.gpsimd.memset(trimask, 1.0)
        nc.gpsimd.affine_select(
            out=trimask,
            in_=trimask,
            compare_op=mybir.AluOpType.is_ge,
            fill=0.0,
            base=0,
            pattern=[[-1, P]],
            channel_multiplier=1,
        )
        trimaskT = consts.tile([P, P], BF16)
        nc.gpsimd.memset(trimaskT, 1.0)
        # keep where s<=q in [s(part), q(free)] layout: (q - s) >= 0
        nc.gpsimd.affine_select(
            out=trimaskT,
            in_=trimaskT,
            compare_op=mybir.AluOpType.is_ge,
            fill=0.0,
            base=0,
            pattern=[[1, P]],
            channel_multiplier=-1,
        )
        bias_ln3eps = consts.tile([P, 1], F32)
        nc.vector.memset(bias_ln3eps, 1e-5 * LN3)
        d2t = []
        for i in range(NB):
            t = consts.tile([P, L], F32, tag=f"d2_{i}")
            nc.gpsimd.dma_start(out=t, in_=d2[i * P : (i + 1) * P, :])
            d2t.append(t)

        # ---- sigma prologue: chains for all batches, both layouts ----
        # normal layout: per (b, L-block) [128, H]; packed: per b [128, 4, H]
        raw_n, raw_p = [], []
        for b in range(BL):
            rp = sparams.tile([P, RPP, H], F32, tag=f"rawp{b}")
            nc.gpsimd.dma_start(out=rp, in_=sgp[b])
            raw_p.append(rp)
            row = []
            for t in range(NB):
                rn = sparams.tile([P, H], F32, tag=f"rawn{b}_{t}")
                nc.gpsimd.dma_start(out=rn, in_=sg[b, t * P : (t + 1) * P, :])
                row.append(rn)
            raw_n.append(row)

        def sig_chain(dst_sig, src):
            s1 = small.tile(list(src.shape), F32, tag="s1")
            nc.scalar.activation(out=s1, in_=src, func=ACT.Sigmoid, scale=5.0)
            u = small.tile(list(src.shape), F32, tag="u")
            nc.scalar.activation(
                out=u, in_=s1, func=ACT.Exp, scale=LN3, bias=bias_ln3eps
            )
            nc.vector.tensor_scalar_add(out=dst_sig, in0=u, scalar1=-1.0)

        sig_t = [[None] * NB for _ in range(BL)]
        ns_t = [[None] * NB for _ in range(BL)]
        lc_t = [[None] * NB for _ in range(BL)]
        sig4 = [None] * BL
        for b in range(BL):
            s4 = sparams.tile([P, RPP, H], F32, tag=f"sig4_{b}")
            sig_chain(s4, raw_p[b])
            sig4[b] = s4
            for t in range(NB):
                s = sparams.tile([P, H], F32, tag=f"sig{b}_{t}")
                sig_chain(s, raw_n[b][t])
                sig_t[b][t] = s
                sq = small.tile([P, H], F32, tag="sq")
                nc.vector.tensor_mul(out=sq, in0=s, in1=s)
                rq = small.tile([P, H], F32, tag="rq")
                nc.vector.reciprocal(out=rq, in_=sq)
                ns = sparams.tile([P, H], F32, tag=f"ns{b}_{t}")
                nc.vector.tensor_scalar_mul(out=ns, in0=rq, scalar1=-0.5)
                ns_t[b][t] = ns
                ln = small.tile([P, H], F32, tag="ln")
                nc.scalar.activation(out=ln, in_=s, func=ACT.Ln)
                lc = sparams.tile([P, H], F32, tag=f"lc{b}_{t}")
                nc.vector.tensor_scalar(
                    out=lc,
                    in0=ln,
                    scalar1=-1.0,
                    scalar2=NEG_HALF_LN_2PI,
                    op0=MUL,
                    op1=mybir.AluOpType.add,
                )
                lc_t[b][t] = lc

        for b in range(BL):
            # ---- load pre-transposed Q/K and ones-augmented V (bf16) ----
            # SWDGE (gpsimd) so loads never queue behind the output stream.
            qT, kT = [], []
            for hp in range(HP):
                qTt = slabs.tile([P, L], BF16, tag=f"qT{hp}")
                nc.gpsimd.dma_start(out=qTt, in_=qt[b, hp])
                qT.append(qTt)
                kTt = slabs.tile([P, L], BF16, tag=f"kT{hp}")
                nc.gpsimd.dma_start(out=kTt, in_=kt[b, hp])
                kT.append(kTt)
            vb = []
            for t in range(NB):
                vt = slabs.tile([P, H * E1], BF16, tag=f"v{t}")
                nc.gpsimd.dma_start(out=vt, in_=v5[b, t * P : (t + 1) * P, :])
                vb.append(vt)

            # ---- sigma_out: 4 rows per partition -> 8 KiB DMA runs ----
            for h in range(H):
                sgo = gop.tile([P, RPP, L], F32, tag="sgo")
                nc.vector.tensor_copy(out=sgo, in_=_bcast(sig4[b][:, :, h], L))
                eng = nc.sync if h % 2 == 0 else nc.scalar
                eng.dma_start(
                    out=go[b, h].rearrange("(p k) s -> p (k s)", k=RPP),
                    in_=sgo,
                )

            # ---- per (row-block, head) attention + prior ----
            # series rows beyond the causal width are never written — the
            # runner donates pre-zeroed output buffers, so they stay 0.
            for i in range(NB):
                W = (i + 1) * P  # causal width of this row block
                rows = slice(i * P, (i + 1) * P)
                vos_all = work.tile([P, H * E], F32, tag="voall")
                for h in range(H):
                    hp, ho = h // 2, (h % 2) * E
                    sc = ps_sc.tile([P, L], F32, tag="sc")
                    nc.tensor.matmul(
                        sc[:, :W],
                        lhsT=qT[hp][ho : ho + E, rows],
                        rhs=kT[hp][ho : ho + E, :W],
                        start=True,
                        stop=True,
                    )
                    expb = work.tile([P, L], BF16, tag="exp")
                    nc.scalar.activation(
                        out=expb[:, :W], in_=sc[:, :W], func=ACT.Exp, scale=0.125
                    )
                    # series @ [V | 1]: transposed exp blocks; col E is the
                    # row-sum. The diagonal chunk's upper triangle is zeroed
                    # during the PSUM->SBUF copy (mask folded into the mul).
                    eTs = []
                    for j in range(i + 1):
                        pt = ps_tr.tile([P, P], BF16, tag="ps_tr")
                        nc.tensor.transpose(pt, expb[:, j * P : (j + 1) * P], ident)
                        eT = eTp.tile([P, P], BF16, tag="eT")
                        if j == i:
                            nc.vector.tensor_mul(out=eT, in0=pt, in1=trimaskT)
                        else:
                            nc.vector.tensor_copy(out=eT, in_=pt)
                        eTs.append(eT)
                    va = ps_av.tile([P, E1], F32, tag="va")
                    for j in range(i + 1):
                        nc.tensor.matmul(
                            va,
                            lhsT=eTs[j],
                            rhs=vb[j][:, h * E1 : (h + 1) * E1],
                            start=(j == 0),
                            stop=(j == i),
                        )
                    rinv = small.tile([P, 1], F32, tag="rinv")
                    nc.vector.reciprocal(out=rinv, in_=va[:, E : E + 1])
                    serf = work.tile([P, L], F32, tag="ser")
                    if i > 0:
                        nc.vector.tensor_scalar_mul(
                            out=serf[:, : i * P], in0=expb[:, : i * P], scalar1=rinv
                        )
                    nc.vector.scalar_tensor_tensor(
                        out=serf[:, i * P : W],
                        in0=expb[:, i * P : W],
                        scalar=rinv,
                        in1=trimask,
                        op0=MUL,
                        op1=MUL,
                    )
                    nc.sync.dma_start(out=so[b, h, rows, :W], in_=serf[:, :W])
                    nc.vector.tensor_scalar_mul(
                        out=vos_all[:, h * E : (h + 1) * E],
                        in0=va[:, :E],
                        scalar1=rinv,
                    )
                    # prior: exp(d2 * (-1/(2 sig^2)) + ln c) in one ACT pass
                    pri = work.tile([P, L], F32, tag="pri")
                    nc.scalar.activation(
                        out=pri,
                        in_=d2t[i],
                        func=ACT.Exp,
                        scale=ns_t[b][i][:, h : h + 1],
                        bias=lc_t[b][i][:, h : h + 1],
                    )
                    nc.scalar.dma_start(out=po[b, h, rows, :], in_=pri)
                nc.sync.dma_start(out=vo[b, rows, :], in_=vos_all)
    return _split_excess_waits(nc)


_nc_cache = None
last_results = None


def kernel(queries, keys, values, sigma, attention_mask=None, **_unused):
    """Full-input entry point: shard over 8 cores, run, gather."""
    global _nc_cache, last_results
    if _nc_cache is None:
        _nc_cache = _build()
    nc = _nc_cache

    queries = np.ascontiguousarray(np.asarray(queries), dtype=np.float32)
    keys = np.ascontiguousarray(np.asarray(keys), dtype=np.float32)
    values = np.ascontiguousarray(np.asarray(values), dtype=np.float32)
    sigma = np.ascontiguousarray(np.asarray(sigma), dtype=np.float32)

    bf = ml_dtypes.bfloat16
    # Q/K transposed to [B, head-pair, 2E, L] so two heads' E dims stack on
    # the 128 SBUF partitions; V gets a ones column per head (row-sum trick).
    qT = np.ascontiguousarray(
        queries.reshape(B, L, HP, 2 * E).transpose(0, 2, 3, 1)
    ).astype(bf)
    kT = np.ascontiguousarray(
        keys.reshape(B, L, HP, 2 * E).transpose(0, 2, 3, 1)
    ).astype(bf)
    v5 = np.ones((B, L, H, E1), dtype=bf)
    v5[..., :E] = values.reshape(B, L, H, E).astype(bf)
    v5 = v5.reshape(B, L, H * E1)
    # packed sigma: sgp[b, p, k, h] = sigma[b, 4p+k, h]
    sgp = sigma.reshape(B, P, RPP, H)

    idx = np.arange(L, dtype=np.float32)
    d2 = (idx[:, None] - idx[None, :]) ** 2

    in_maps = []
    for c in range(N_CORES):
        bs = slice(c * BL, (c + 1) * BL)
        in_maps.append(
            {
                "qt": qT[bs],
                "kt": kT[bs],
                "v5": v5[bs],
                "sg": sigma[bs],
                "sgp": sgp[bs],
                "d2": d2,
            }
        )

    res = run_bass_kernel_spmd(nc, in_maps, core_ids=list(range(N_CORES)))
    last_results = res

    V = np.concatenate(
        [res.results[c]["vo"].reshape(BL, L, H, E) for c in range(N_CORES)], axis=0
    )
    series = np.concatenate([res.results[c]["so"] for c in range(N_CORES)], axis=0)
    prior = np.concatenate([res.results[c]["po"] for c in range(N_CORES)], axis=0)
    sigma_out = np.concatenate([res.results[c]["go"] for c in range(N_CORES)], axis=0)
    return V, series, prior, sigma_out
